# revision 19
# baseline (speedup 1.0000x reference)
"""AdditiveAttention2D (Bahdanau-style) on 8 Trainium2 NeuronCores.

Reference (per batch b):
    sW = s @ W, hU = h @ U                              [L, D]
    scores[l, m] = sum_d v[d] * tanh(sW[l, d] + hU[m, d])
    attn = softmax_m(scores);  out = attn @ h           [L, D]

Sharding: the B*L = 1024 query rows split across 8 cores (128 rows each,
each core's rows inside one batch). Each core gets its batch's full h
(keys/values) plus replicated W, U, v. No collectives; the host
concatenates the per-core output shards. The host also pre-lays-out the
shards (transposes, bf16 casts, the v-diagonal weight tile) — pure
layout, no FLOPs.

Per-core layout: d lives on partitions. For query q the tanh argument is
hU_T[d, m] + sW_T[d, q] — a per-partition-scalar broadcast add (DVE, 4x
bf16 mode), a bulk fused tanh (ScalarE — the bottleneck engine, ~1
elem/lane/cycle), and the v-weighted d-reduction as a PE matmul with v
embedded in column j of a [D, G] stationary tile so query j lands in
PSUM partition j. Softmax skips max-subtraction (|scores| <= ||v||_1 ~ 9
since |tanh| < 1, so exp cannot overflow fp32). exp-scores go through a
PE transpose to become the stationary operand of the attn @ h
accumulation, then rows are scaled by 1/rowsum.

The 16-matmul bursts after each half-chunk tanh are long enough
(~4.5us) to flip the PE HAM clock gate to 2.4 GHz and the inter-burst
gaps are short enough (<3.4us) to keep it there.
"""

from contextlib import ExitStack

import ml_dtypes
import numpy as np

import concourse.bass as bass
import concourse.mybir as mybir
import concourse.tile as tile
from concourse import bacc
from concourse.bass_utils import run_bass_kernel_spmd

F32 = mybir.dt.float32
BF16 = mybir.dt.bfloat16
AF = mybir.ActivationFunctionType

B, L, D = 2, 512, 128
N_CORES = 8
QPC = B * L // N_CORES  # query rows per core (128)
G = 32                  # queries per chunk (softmax granularity)
SUB = 8                 # queries per tanh/matmul burst (FD=4096 is ACT's sweet spot)
NSUB = G // SUB         # bursts per chunk (4)
NCH = QPC // G          # chunks per core (4)
MT = L // 128           # 128-row key tiles per batch (4)


def build_nc() -> bass.Bass:
    # Bacc (not plain Bass): its compile() runs move_matmul_waits_to_ldweights
    # + generate_event_semaphores, which legalize multi-sem waits down to the
    # 1-wait-per-instruction limit this walrus enforces.
    nc = bacc.Bacc()
    F32R = mybir.dt.float32r
    sT_d = nc.declare_dram_parameter("sT", [D, QPC], F32R, isOutput=False)
    hT_d = nc.declare_dram_parameter("hT", [D, L], F32R, isOutput=False)
    hb_d = nc.declare_dram_parameter("hb", [128, MT, D], BF16, isOutput=False)
    W_d = nc.declare_dram_parameter("W", [D, D], F32R, isOutput=False)
    U_d = nc.declare_dram_parameter("U", [D, D], F32R, isOutput=False)
    vm_d = nc.declare_dram_parameter("vmat", [D, G, G], BF16, isOutput=False)
    id_d = nc.declare_dram_parameter("ident", [G, G], BF16, isOutput=False)
    o_d = nc.declare_dram_parameter("out", [QPC, D], F32, isOutput=True)

    with ExitStack() as ctx:
        tc = ctx.enter_context(tile.TileContext(nc))
        consts = ctx.enter_context(tc.tile_pool(name="consts", bufs=1))
        xpool = ctx.enter_context(tc.tile_pool(name="x", bufs=2))
        tpool = ctx.enter_context(tc.tile_pool(name="t", bufs=2))
        spool = ctx.enter_context(tc.tile_pool(name="small", bufs=3))

        # ---------------- prologue ----------------
        # Critical-path loads (hT, U gate the hU matmul -> adds -> 1st tanh)
        # go first on the sync HWDGE queue; bulk non-critical loads go via
        # gpsimd SWDGE so their ~0.6us issue cost doesn't delay the former.
        hT_sb = consts.tile([D, L], F32R)
        for t in range(2):
            nc.sync.dma_start(
                out=hT_sb[:, bass.ts(t, 256)], in_=hT_d[:, bass.ts(t, 256)]
            )
        U_sb = consts.tile([D, D], F32R)
        nc.scalar.dma_start(out=U_sb, in_=U_d[:, :])
        W_sb = consts.tile([D, D], F32R)
        nc.scalar.dma_start(out=W_sb, in_=W_d[:, :])
        sT_sb = consts.tile([D, QPC], F32R)
        nc.scalar.dma_start(out=sT_sb, in_=sT_d[:, :])
        vmat = consts.tile([D, G, G], BF16)
        nc.gpsimd.dma_start(out=vmat, in_=vm_d[:, :, :])
        hb_sb = consts.tile([128, MT, D], BF16)
        nc.gpsimd.dma_start(out=hb_sb, in_=hb_d[:, :, :])
        ident = consts.tile([G, G], BF16)
        nc.gpsimd.dma_start(out=ident, in_=id_d[:, :])

        hU_sb = consts.tile([D, L], BF16)
        sW_sb = consts.tile([D, QPC], F32)

        with tc.tile_pool(name="pp_pro", bufs=2, space="PSUM") as pp_pro:
            # hU_T[dout, m] = sum_din U[din, dout] * hT[din, m]. float32r
            # streams fp32 data through the PE at 1 cycle/row (vs fp32's
            # two half-speed passes) with TF32-ish multiply precision —
            # plenty here since X is consumed in bf16 anyway.
            hU_ps = pp_pro.tile([D, L], F32, tag="pro")
            nc.tensor.matmul(hU_ps, U_sb, hT_sb, start=True, stop=True)
            nc.vector.tensor_copy(hU_sb, hU_ps)
            sW_ps = pp_pro.tile([D, QPC], F32, tag="pro")
            nc.tensor.matmul(sW_ps, W_sb, sT_sb, start=True, stop=True)
            nc.vector.tensor_copy(sW_sb, sW_ps)

        pp = ctx.enter_context(tc.tile_pool(name="pp", bufs=2, space="PSUM"))

        # ---------------- main loop (software-pipelined emission) --------
        # Per-engine steady-state orders (c = chunk):
        #   ACT: ... tanh(c+1,h0) tanh(c+1,h1) exp(c) ...
        #   DVE: ... adds(c+2) [reduce/recip/eTcopy/scale](c) ...
        #   PE : ... mms(c+1,h0) mms(c+1,h1) [transp/attn](c) ...
        sc_tiles: dict[int, object] = {}
        exp_tiles: dict[int, object] = {}

        def stage_a(c):
            """adds + tanh + v-reduction matmuls for chunk c."""
            sc = pp.tile([G, L], F32, tag="scores")
            for sub in range(NSUB):
                j0 = sub * SUB
                X = xpool.tile([D, SUB, L], BF16, tag=f"X{sub % 2}")
                for j in range(j0, j0 + SUB):
                    q = c * G + j
                    nc.vector.tensor_scalar_add(
                        X[:, j - j0, :], hU_sb, sW_sb[:, q : q + 1]
                    )
                T = tpool.tile([D, SUB, L], BF16, tag=f"T{sub % 2}")
                nc.scalar.activation(
                    T.rearrange("p a b -> p (a b)"),
                    X.rearrange("p a b -> p (a b)"),
                    AF.Tanh,
                )
                for j in range(j0, j0 + SUB):
                    nc.tensor.matmul(
                        sc,
                        vmat[:, j, :],
                        T[:, j - j0, :],
                        start=(j == 0),
                        stop=(j == G - 1),
                    )
            sc_tiles[c] = sc

        def stage_exp(c):
            """exp for chunk c — emitted after tanh(c+1)."""
            exp_sb = spool.tile([G, L], BF16, tag="exp")
            nc.scalar.activation(exp_sb, sc_tiles.pop(c), AF.Exp)
            exp_tiles[c] = exp_sb

        def stage_tail(c):
            """softmax-normalize + attn @ h + store for chunk c."""
            exp_sb = exp_tiles.pop(c)
            sums = spool.tile([G, 1], F32, tag="sums")
            nc.vector.tensor_reduce(
                sums, exp_sb, axis=mybir.AxisListType.X, op=mybir.AluOpType.add
            )
            recip = spool.tile([G, 1], F32, tag="recip")
            nc.vector.reciprocal(recip, sums)
            eT_ps = pp.tile([128, MT, G], BF16, tag="eT")
            for t in range(MT):
                nc.tensor.transpose(
                    eT_ps[:, t, :], exp_sb[:, t * 128 : (t + 1) * 128], ident
                )
            eT_sb = spool.tile([128, MT, G], BF16, tag="eTs")
            nc.vector.tensor_copy(eT_sb, eT_ps)
            at_ps = pp.tile([G, D], F32, tag="attn")
            for t in range(MT):
                nc.tensor.matmul(
                    at_ps,
                    eT_sb[:, t, :],
                    hb_sb[:, t, :],
                    start=(t == 0),
                    stop=(t == MT - 1),
                )
            out_sb = spool.tile([G, D], F32, tag="out")
            nc.vector.tensor_scalar_mul(out_sb, at_ps, recip[:, 0:1])
            nc.sync.dma_start(out=o_d[bass.ts(c, G), :], in_=out_sb)

        stage_a(0)
        if NCH > 1:
            stage_a(1)
        for c in range(NCH):
            stage_exp(c)
            if c + 2 < NCH:
                stage_a(c + 2)
            stage_tail(c)

    nc.compile()
    return nc


_NC_CACHE: list = []


def _get_nc() -> bass.Bass:
    if not _NC_CACHE:
        _NC_CACHE.append(build_nc())
    return _NC_CACHE[0]


def _make_in_maps(s, h, W, U, v):
    s2 = np.ascontiguousarray(np.asarray(s, np.float32).reshape(B * L, D))
    h2 = np.asarray(h, np.float32)
    W2 = np.ascontiguousarray(np.asarray(W, np.float32))
    U2 = np.ascontiguousarray(np.asarray(U, np.float32))
    v2 = np.asarray(v, np.float32)
    vmat = np.zeros((D, G, G), np.float32)
    for j in range(G):
        vmat[:, j, j] = v2[:, 0]
    vmat = vmat.astype(ml_dtypes.bfloat16)
    ident = np.eye(G, dtype=ml_dtypes.bfloat16)
    in_maps = []
    for c in range(N_CORES):
        b = c * QPC // L
        h_b = h2[b]  # [L, D]
        hb = np.ascontiguousarray(
            h_b.reshape(MT, 128, D).transpose(1, 0, 2).astype(ml_dtypes.bfloat16)
        )
        in_maps.append(
            {
                "sT": np.ascontiguousarray(s2[c * QPC : (c + 1) * QPC].T),
                "hT": np.ascontiguousarray(h_b.T),
                "hb": hb,
                "W": W2,
                "U": U2,
                "vmat": vmat,
                "ident": ident,
            }
        )
    return in_maps


def run_spmd(s, h, W, U, v, **kwargs):
    """Run the kernel on 8 cores; returns the BassKernelResults."""
    nc = _get_nc()
    in_maps = _make_in_maps(s, h, W, U, v)
    return run_bass_kernel_spmd(nc, in_maps, core_ids=list(range(N_CORES)), **kwargs)


def kernel(s, h, W, U, v):
    res = run_spmd(s, h, W, U, v)
    shards = [np.asarray(res.results[c]["out"]) for c in range(N_CORES)]
    return np.concatenate(shards, axis=0).reshape(B, L, D).astype(np.float32)


# revision 20
# speedup vs baseline: 1.0028x; 1.0028x over previous
"""AdditiveAttention2D (Bahdanau-style) on 8 Trainium2 NeuronCores.

Reference (per batch b):
    sW = s @ W, hU = h @ U                              [L, D]
    scores[l, m] = sum_d v[d] * tanh(sW[l, d] + hU[m, d])
    attn = softmax_m(scores);  out = attn @ h           [L, D]

Sharding: the B*L = 1024 query rows split across 8 cores (128 rows each,
each core's rows inside one batch). Each core gets its batch's full h
(keys/values) plus replicated W, U, v. No collectives; the host
concatenates the per-core output shards. The host also pre-lays-out the
shards (transposes, bf16 casts, the v-diagonal weight tile) — pure
layout, no FLOPs.

Per-core layout: d lives on partitions. For query q the tanh argument is
hU_T[d, m] + sW_T[d, q] — a per-partition-scalar broadcast add (DVE, 4x
bf16 mode), a bulk fused tanh (ScalarE — the bottleneck engine, ~1
elem/lane/cycle), and the v-weighted d-reduction as a PE matmul with v
embedded in column j of a [D, G] stationary tile so query j lands in
PSUM partition j. Softmax skips max-subtraction (|scores| <= ||v||_1 ~ 9
since |tanh| < 1, so exp cannot overflow fp32). exp-scores go through a
PE transpose to become the stationary operand of the attn @ h
accumulation, then rows are scaled by 1/rowsum.

The 16-matmul bursts after each half-chunk tanh are long enough
(~4.5us) to flip the PE HAM clock gate to 2.4 GHz and the inter-burst
gaps are short enough (<3.4us) to keep it there.
"""

from contextlib import ExitStack

import ml_dtypes
import numpy as np

import concourse.bass as bass
import concourse.mybir as mybir
import concourse.tile as tile
from concourse import bacc
from concourse.bass_utils import run_bass_kernel_spmd

F32 = mybir.dt.float32
BF16 = mybir.dt.bfloat16
AF = mybir.ActivationFunctionType

B, L, D = 2, 512, 128
N_CORES = 8
QPC = B * L // N_CORES  # query rows per core (128)
G = 32                  # queries per chunk (softmax granularity)
SUB = 8                 # queries per tanh/matmul burst (FD=4096 is ACT's sweet spot)
NSUB = G // SUB         # bursts per chunk (4)
NCH = QPC // G          # chunks per core (4)
MT = L // 128           # 128-row key tiles per batch (4)


def build_nc() -> bass.Bass:
    # Bacc (not plain Bass): its compile() runs move_matmul_waits_to_ldweights
    # + generate_event_semaphores, which legalize multi-sem waits down to the
    # 1-wait-per-instruction limit this walrus enforces.
    nc = bacc.Bacc()
    F32R = mybir.dt.float32r
    sT_d = nc.declare_dram_parameter("sT", [D, QPC], F32R, isOutput=False)
    hT_d = nc.declare_dram_parameter("hT", [D, L], F32R, isOutput=False)
    hb_d = nc.declare_dram_parameter("hb", [128, MT, D], BF16, isOutput=False)
    W_d = nc.declare_dram_parameter("W", [D, D], F32R, isOutput=False)
    U_d = nc.declare_dram_parameter("U", [D, D], F32R, isOutput=False)
    vm_d = nc.declare_dram_parameter("vmat", [D, G, G], BF16, isOutput=False)
    id_d = nc.declare_dram_parameter("ident", [G, G], BF16, isOutput=False)
    o_d = nc.declare_dram_parameter("out", [QPC, D], F32, isOutput=True)

    with ExitStack() as ctx:
        tc = ctx.enter_context(tile.TileContext(nc))
        consts = ctx.enter_context(tc.tile_pool(name="consts", bufs=1))
        xpool = ctx.enter_context(tc.tile_pool(name="x", bufs=2))
        tpool = ctx.enter_context(tc.tile_pool(name="t", bufs=2))
        spool = ctx.enter_context(tc.tile_pool(name="small", bufs=3))

        # ---------------- prologue ----------------
        # Critical-path loads (hT, U gate the hU matmul -> adds -> 1st tanh)
        # go first on the sync HWDGE queue; bulk non-critical loads go via
        # gpsimd SWDGE so their ~0.6us issue cost doesn't delay the former.
        hT_sb = consts.tile([D, L], F32R)
        for t in range(2):
            nc.sync.dma_start(
                out=hT_sb[:, bass.ts(t, 256)], in_=hT_d[:, bass.ts(t, 256)]
            )
        U_sb = consts.tile([D, D], F32R)
        nc.sync.dma_start(out=U_sb, in_=U_d[:, :])
        W_sb = consts.tile([D, D], F32R)
        nc.sync.dma_start(out=W_sb, in_=W_d[:, :])
        sT_sb = consts.tile([D, QPC], F32R)
        nc.sync.dma_start(out=sT_sb, in_=sT_d[:, :])
        vmat = consts.tile([D, G, G], BF16)
        nc.gpsimd.dma_start(out=vmat, in_=vm_d[:, :, :])
        hb_sb = consts.tile([128, MT, D], BF16)
        nc.gpsimd.dma_start(out=hb_sb, in_=hb_d[:, :, :])
        ident = consts.tile([G, G], BF16)
        nc.gpsimd.dma_start(out=ident, in_=id_d[:, :])

        hU_sb = consts.tile([D, L], BF16)
        sW_sb = consts.tile([D, QPC], F32)

        with tc.tile_pool(name="pp_pro", bufs=2, space="PSUM") as pp_pro:
            # hU_T[dout, m] = sum_din U[din, dout] * hT[din, m]. float32r
            # streams fp32 data through the PE at 1 cycle/row (vs fp32's
            # two half-speed passes) with TF32-ish multiply precision —
            # plenty here since X is consumed in bf16 anyway.
            hU_ps = pp_pro.tile([D, L], F32, tag="pro")
            nc.tensor.matmul(hU_ps, U_sb, hT_sb, start=True, stop=True)
            nc.vector.tensor_copy(hU_sb, hU_ps)
            sW_ps = pp_pro.tile([D, QPC], F32, tag="pro")
            nc.tensor.matmul(sW_ps, W_sb, sT_sb, start=True, stop=True)
            nc.vector.tensor_copy(sW_sb, sW_ps)

        pp = ctx.enter_context(tc.tile_pool(name="pp", bufs=2, space="PSUM"))

        # ---------------- main loop (software-pipelined emission) --------
        # Per-engine steady-state orders (c = chunk):
        #   ACT: ... tanh(c+1,h0) tanh(c+1,h1) exp(c) ...
        #   DVE: ... adds(c+2) [reduce/recip/eTcopy/scale](c) ...
        #   PE : ... mms(c+1,h0) mms(c+1,h1) [transp/attn](c) ...
        sc_tiles: dict[int, object] = {}
        exp_tiles: dict[int, object] = {}

        def stage_a(c):
            """adds + tanh + v-reduction matmuls for chunk c."""
            sc = pp.tile([G, L], F32, tag="scores")
            for sub in range(NSUB):
                j0 = sub * SUB
                X = xpool.tile([D, SUB, L], BF16, tag=f"X{sub % 2}")
                for j in range(j0, j0 + SUB):
                    q = c * G + j
                    nc.vector.tensor_scalar_add(
                        X[:, j - j0, :], hU_sb, sW_sb[:, q : q + 1]
                    )
                T = tpool.tile([D, SUB, L], BF16, tag=f"T{sub % 2}")
                nc.scalar.activation(
                    T.rearrange("p a b -> p (a b)"),
                    X.rearrange("p a b -> p (a b)"),
                    AF.Tanh,
                )
                for j in range(j0, j0 + SUB):
                    nc.tensor.matmul(
                        sc,
                        vmat[:, j, :],
                        T[:, j - j0, :],
                        start=(j == 0),
                        stop=(j == G - 1),
                    )
            sc_tiles[c] = sc

        def stage_exp(c):
            """exp for chunk c — emitted after tanh(c+1)."""
            exp_sb = spool.tile([G, L], BF16, tag="exp")
            nc.scalar.activation(exp_sb, sc_tiles.pop(c), AF.Exp)
            exp_tiles[c] = exp_sb

        def stage_tail(c):
            """softmax-normalize + attn @ h + store for chunk c."""
            exp_sb = exp_tiles.pop(c)
            sums = spool.tile([G, 1], F32, tag="sums")
            nc.vector.tensor_reduce(
                sums, exp_sb, axis=mybir.AxisListType.X, op=mybir.AluOpType.add
            )
            recip = spool.tile([G, 1], F32, tag="recip")
            nc.vector.reciprocal(recip, sums)
            eT_ps = pp.tile([128, MT, G], BF16, tag="eT")
            for t in range(MT):
                nc.tensor.transpose(
                    eT_ps[:, t, :], exp_sb[:, t * 128 : (t + 1) * 128], ident
                )
            eT_sb = spool.tile([128, MT, G], BF16, tag="eTs")
            nc.vector.tensor_copy(eT_sb, eT_ps)
            at_ps = pp.tile([G, D], F32, tag="attn")
            for t in range(MT):
                nc.tensor.matmul(
                    at_ps,
                    eT_sb[:, t, :],
                    hb_sb[:, t, :],
                    start=(t == 0),
                    stop=(t == MT - 1),
                )
            out_sb = spool.tile([G, D], F32, tag="out")
            nc.vector.tensor_scalar_mul(out_sb, at_ps, recip[:, 0:1])
            nc.sync.dma_start(out=o_d[bass.ts(c, G), :], in_=out_sb)

        stage_a(0)
        if NCH > 1:
            stage_a(1)
        for c in range(NCH):
            stage_exp(c)
            if c + 2 < NCH:
                stage_a(c + 2)
            stage_tail(c)

    nc.compile()
    return nc


_NC_CACHE: list = []


def _get_nc() -> bass.Bass:
    if not _NC_CACHE:
        _NC_CACHE.append(build_nc())
    return _NC_CACHE[0]


def _make_in_maps(s, h, W, U, v):
    s2 = np.ascontiguousarray(np.asarray(s, np.float32).reshape(B * L, D))
    h2 = np.asarray(h, np.float32)
    W2 = np.ascontiguousarray(np.asarray(W, np.float32))
    U2 = np.ascontiguousarray(np.asarray(U, np.float32))
    v2 = np.asarray(v, np.float32)
    vmat = np.zeros((D, G, G), np.float32)
    for j in range(G):
        vmat[:, j, j] = v2[:, 0]
    vmat = vmat.astype(ml_dtypes.bfloat16)
    ident = np.eye(G, dtype=ml_dtypes.bfloat16)
    in_maps = []
    for c in range(N_CORES):
        b = c * QPC // L
        h_b = h2[b]  # [L, D]
        hb = np.ascontiguousarray(
            h_b.reshape(MT, 128, D).transpose(1, 0, 2).astype(ml_dtypes.bfloat16)
        )
        in_maps.append(
            {
                "sT": np.ascontiguousarray(s2[c * QPC : (c + 1) * QPC].T),
                "hT": np.ascontiguousarray(h_b.T),
                "hb": hb,
                "W": W2,
                "U": U2,
                "vmat": vmat,
                "ident": ident,
            }
        )
    return in_maps


def run_spmd(s, h, W, U, v, **kwargs):
    """Run the kernel on 8 cores; returns the BassKernelResults."""
    nc = _get_nc()
    in_maps = _make_in_maps(s, h, W, U, v)
    return run_bass_kernel_spmd(nc, in_maps, core_ids=list(range(N_CORES)), **kwargs)


def kernel(s, h, W, U, v):
    res = run_spmd(s, h, W, U, v)
    shards = [np.asarray(res.results[c]["out"]) for c in range(N_CORES)]
    return np.concatenate(shards, axis=0).reshape(B, L, D).astype(np.float32)


# revision 23
# speedup vs baseline: 1.1849x; 1.1816x over previous
"""AdditiveAttention2D (Bahdanau-style) on 8 Trainium2 NeuronCores.

Reference (per batch b):
    sW = s @ W, hU = h @ U                              [L, D]
    scores[l, m] = sum_d v[d] * tanh(sW[l, d] + hU[m, d])
    attn = softmax_m(scores);  out = attn @ h           [L, D]

Sharding: the B*L = 1024 query rows split across 8 cores (128 rows each,
each core's rows inside one batch). Each core gets its batch's full h
(keys/values) plus replicated W, U, v. No collectives; the host
concatenates the per-core output shards. The host also pre-lays-out the
shards (transposes, bf16 casts, the v-diagonal weight tile) — pure
layout, no FLOPs.

Per-core layout: d lives on partitions. For query q the tanh argument is
hU_T[d, m] + sW_T[d, q] — a per-partition-scalar broadcast add (DVE, 4x
bf16 mode), a bulk fused tanh (ScalarE — the bottleneck engine, ~1
elem/lane/cycle), and the v-weighted d-reduction as a PE matmul with v
embedded in column j of a [D, G] stationary tile so query j lands in
PSUM partition j. Softmax skips max-subtraction (|scores| <= ||v||_1 ~ 9
since |tanh| < 1, so exp cannot overflow fp32). exp-scores go through a
PE transpose to become the stationary operand of the attn @ h
accumulation, then rows are scaled by 1/rowsum.

The 16-matmul bursts after each half-chunk tanh are long enough
(~4.5us) to flip the PE HAM clock gate to 2.4 GHz and the inter-burst
gaps are short enough (<3.4us) to keep it there.
"""

from contextlib import ExitStack

import ml_dtypes
import numpy as np

import concourse.bass as bass
import concourse.mybir as mybir
import concourse.tile as tile
from concourse import bacc
from concourse.bass_utils import run_bass_kernel_spmd

F32 = mybir.dt.float32
BF16 = mybir.dt.bfloat16
AF = mybir.ActivationFunctionType

B, L, D = 2, 512, 128
N_CORES = 8
QPC = B * L // N_CORES  # query rows per core (128)
G = 32                  # queries per chunk (softmax granularity)
NCH = QPC // G          # chunks per core (4)
MT = L // 128           # 128-row key tiles per batch (4)


def build_nc() -> bass.Bass:
    # Bacc (not plain Bass): its compile() runs move_matmul_waits_to_ldweights
    # + generate_event_semaphores, which legalize multi-sem waits down to the
    # 1-wait-per-instruction limit this walrus enforces.
    nc = bacc.Bacc()
    F32R = mybir.dt.float32r
    sT_d = nc.declare_dram_parameter("sT", [D, QPC], F32R, isOutput=False)
    hT_d = nc.declare_dram_parameter("hT", [D, L], F32R, isOutput=False)
    hb_d = nc.declare_dram_parameter("hb", [128, MT, D], BF16, isOutput=False)
    W_d = nc.declare_dram_parameter("W", [D, D], F32R, isOutput=False)
    U_d = nc.declare_dram_parameter("U", [D, D], F32R, isOutput=False)
    vm_d = nc.declare_dram_parameter("vmat", [D, G, G], BF16, isOutput=False)
    id_d = nc.declare_dram_parameter("ident", [G, G], BF16, isOutput=False)
    o_d = nc.declare_dram_parameter("out", [QPC, D], F32, isOutput=True)

    with ExitStack() as ctx:
        tc = ctx.enter_context(tile.TileContext(nc))
        consts = ctx.enter_context(tc.tile_pool(name="consts", bufs=1))
        xpool = ctx.enter_context(tc.tile_pool(name="x", bufs=2))
        tpool = ctx.enter_context(tc.tile_pool(name="t", bufs=2))
        spool = ctx.enter_context(tc.tile_pool(name="small", bufs=3))

        # ---------------- prologue ----------------
        # Critical-path loads (hT, U gate the hU matmul -> adds -> 1st tanh)
        # go first on the sync HWDGE queue; bulk non-critical loads go via
        # gpsimd SWDGE so their ~0.6us issue cost doesn't delay the former.
        hT_sb = consts.tile([D, L], F32R)
        nc.sync.dma_start(out=hT_sb, in_=hT_d[:, :])
        U_sb = consts.tile([D, D], F32R)
        nc.sync.dma_start(out=U_sb, in_=U_d[:, :])
        W_sb = consts.tile([D, D], F32R)
        nc.sync.dma_start(out=W_sb, in_=W_d[:, :])
        sT_sb = consts.tile([D, QPC], F32R)
        nc.sync.dma_start(out=sT_sb, in_=sT_d[:, :])
        vmat = consts.tile([D, G, G], BF16)
        nc.gpsimd.dma_start(out=vmat, in_=vm_d[:, :, :])
        hb_sb = consts.tile([128, MT, D], BF16)
        nc.gpsimd.dma_start(out=hb_sb, in_=hb_d[:, :, :])
        ident = consts.tile([G, G], BF16)
        nc.gpsimd.dma_start(out=ident, in_=id_d[:, :])

        hU_sb = consts.tile([D, L], BF16)
        sW_sb = consts.tile([D, QPC], F32)

        with tc.tile_pool(name="pp_pro", bufs=2, space="PSUM") as pp_pro:
            # hU_T[dout, m] = sum_din U[din, dout] * hT[din, m]. float32r
            # streams fp32 data through the PE at 1 cycle/row (vs fp32's
            # two half-speed passes) with TF32-ish multiply precision —
            # plenty here since X is consumed in bf16 anyway.
            hU_ps = pp_pro.tile([D, L], F32, tag="pro")
            nc.tensor.matmul(hU_ps, U_sb, hT_sb, start=True, stop=True)
            nc.vector.tensor_copy(hU_sb, hU_ps)
            sW_ps = pp_pro.tile([D, QPC], F32, tag="pro")
            nc.tensor.matmul(sW_ps, W_sb, sT_sb, start=True, stop=True)
            nc.vector.tensor_copy(sW_sb, sW_ps)

        pp = ctx.enter_context(tc.tile_pool(name="pp", bufs=2, space="PSUM"))

        # ---------------- main loop (software-pipelined emission) --------
        # Per-engine steady-state orders (c = chunk):
        #   ACT: ... tanh(c+1,h0) tanh(c+1,h1) exp(c) ...
        #   DVE: ... adds(c+2) [reduce/recip/eTcopy/scale](c) ...
        #   PE : ... mms(c+1,h0) mms(c+1,h1) [transp/attn](c) ...
        sc_tiles: dict[int, object] = {}
        exp_tiles: dict[int, object] = {}

        def stage_a(c):
            """adds + tanh + v-reduction matmuls for chunk c.

            Chunk 0 uses small (8-query) tanh tiles so the first tanh
            starts as soon as 8 broadcast-adds are done; steady-state
            chunks use 16-query tiles (lower per-instruction overhead).
            """
            subs = [8, 8, 8, 8] if c == 0 else [16, 16]
            sc = pp.tile([G, L], F32, tag="scores")
            j0 = 0
            for si, sub in enumerate(subs):
                X = xpool.tile([D, sub, L], BF16, tag=f"X{si % 2}")
                for j in range(j0, j0 + sub):
                    q = c * G + j
                    nc.vector.tensor_scalar_add(
                        X[:, j - j0, :], hU_sb, sW_sb[:, q : q + 1]
                    )
                T = tpool.tile([D, sub, L], BF16, tag=f"T{si % 2}")
                nc.scalar.activation(
                    T.rearrange("p a b -> p (a b)"),
                    X.rearrange("p a b -> p (a b)"),
                    AF.Tanh,
                )
                for j in range(j0, j0 + sub):
                    nc.tensor.matmul(
                        sc,
                        vmat[:, j, :],
                        T[:, j - j0, :],
                        start=(j == 0),
                        stop=(j == G - 1),
                    )
                j0 += sub
            sc_tiles[c] = sc

        def stage_exp(c):
            """exp for chunk c — emitted after tanh(c+1)."""
            exp_sb = spool.tile([G, L], BF16, tag="exp")
            nc.scalar.activation(exp_sb, sc_tiles.pop(c), AF.Exp)
            exp_tiles[c] = exp_sb

        def stage_tail(c):
            """softmax-normalize + attn @ h + store for chunk c."""
            exp_sb = exp_tiles.pop(c)
            sums = spool.tile([G, 1], F32, tag="sums")
            nc.vector.tensor_reduce(
                sums, exp_sb, axis=mybir.AxisListType.X, op=mybir.AluOpType.add
            )
            recip = spool.tile([G, 1], F32, tag="recip")
            nc.vector.reciprocal(recip, sums)
            eT_ps = pp.tile([128, MT, G], BF16, tag="eT")
            for t in range(MT):
                nc.tensor.transpose(
                    eT_ps[:, t, :], exp_sb[:, t * 128 : (t + 1) * 128], ident
                )
            eT_sb = spool.tile([128, MT, G], BF16, tag="eTs")
            nc.vector.tensor_copy(eT_sb, eT_ps)
            at_ps = pp.tile([G, D], F32, tag="attn")
            for t in range(MT):
                nc.tensor.matmul(
                    at_ps,
                    eT_sb[:, t, :],
                    hb_sb[:, t, :],
                    start=(t == 0),
                    stop=(t == MT - 1),
                )
            out_sb = spool.tile([G, D], F32, tag="out")
            nc.vector.tensor_scalar_mul(out_sb, at_ps, recip[:, 0:1])
            nc.sync.dma_start(out=o_d[bass.ts(c, G), :], in_=out_sb)

        stage_a(0)
        if NCH > 1:
            stage_a(1)
        for c in range(NCH):
            stage_exp(c)
            if c + 2 < NCH:
                stage_a(c + 2)
            stage_tail(c)

    nc.compile()
    return nc


_NC_CACHE: list = []


def _get_nc() -> bass.Bass:
    if not _NC_CACHE:
        _NC_CACHE.append(build_nc())
    return _NC_CACHE[0]


def _make_in_maps(s, h, W, U, v):
    s2 = np.ascontiguousarray(np.asarray(s, np.float32).reshape(B * L, D))
    h2 = np.asarray(h, np.float32)
    W2 = np.ascontiguousarray(np.asarray(W, np.float32))
    U2 = np.ascontiguousarray(np.asarray(U, np.float32))
    v2 = np.asarray(v, np.float32)
    vmat = np.zeros((D, G, G), np.float32)
    for j in range(G):
        vmat[:, j, j] = v2[:, 0]
    vmat = vmat.astype(ml_dtypes.bfloat16)
    ident = np.eye(G, dtype=ml_dtypes.bfloat16)
    in_maps = []
    for c in range(N_CORES):
        b = c * QPC // L
        h_b = h2[b]  # [L, D]
        hb = np.ascontiguousarray(
            h_b.reshape(MT, 128, D).transpose(1, 0, 2).astype(ml_dtypes.bfloat16)
        )
        in_maps.append(
            {
                "sT": np.ascontiguousarray(s2[c * QPC : (c + 1) * QPC].T),
                "hT": np.ascontiguousarray(h_b.T),
                "hb": hb,
                "W": W2,
                "U": U2,
                "vmat": vmat,
                "ident": ident,
            }
        )
    return in_maps


def run_spmd(s, h, W, U, v, **kwargs):
    """Run the kernel on 8 cores; returns the BassKernelResults."""
    nc = _get_nc()
    in_maps = _make_in_maps(s, h, W, U, v)
    return run_bass_kernel_spmd(nc, in_maps, core_ids=list(range(N_CORES)), **kwargs)


def kernel(s, h, W, U, v):
    res = run_spmd(s, h, W, U, v)
    shards = [np.asarray(res.results[c]["out"]) for c in range(N_CORES)]
    return np.concatenate(shards, axis=0).reshape(B, L, D).astype(np.float32)


# revision 25
# speedup vs baseline: 1.2029x; 1.0152x over previous
"""AdditiveAttention2D (Bahdanau-style) on 8 Trainium2 NeuronCores.

Reference (per batch b):
    sW = s @ W, hU = h @ U                              [L, D]
    scores[l, m] = sum_d v[d] * tanh(sW[l, d] + hU[m, d])
    attn = softmax_m(scores);  out = attn @ h           [L, D]

Sharding: the B*L = 1024 query rows split across 8 cores (128 rows each,
each core's rows inside one batch). Each core gets its batch's full h
(keys/values) plus replicated W, U, v. No collectives; the host
concatenates the per-core output shards. The host also pre-lays-out the
shards (transposes, bf16 casts, the v-diagonal weight tile) — pure
layout, no FLOPs.

Per-core layout: d lives on partitions. For query q the tanh argument is
hU_T[d, m] + sW_T[d, q] — a per-partition-scalar broadcast add (DVE, 4x
bf16 mode), a bulk fused tanh (ScalarE — the bottleneck engine, ~1
elem/lane/cycle), and the v-weighted d-reduction as a PE matmul with v
embedded in column j of a [D, G] stationary tile so query j lands in
PSUM partition j. Softmax skips max-subtraction (|scores| <= ||v||_1 ~ 9
since |tanh| < 1, so exp cannot overflow fp32). exp-scores go through a
PE transpose to become the stationary operand of the attn @ h
accumulation, then rows are scaled by 1/rowsum.

The 16-matmul bursts after each half-chunk tanh are long enough
(~4.5us) to flip the PE HAM clock gate to 2.4 GHz and the inter-burst
gaps are short enough (<3.4us) to keep it there.
"""

from contextlib import ExitStack

import ml_dtypes
import numpy as np

import concourse.bass as bass
import concourse.mybir as mybir
import concourse.tile as tile
from concourse import bacc
from concourse.bass_utils import run_bass_kernel_spmd

F32 = mybir.dt.float32
BF16 = mybir.dt.bfloat16
AF = mybir.ActivationFunctionType

B, L, D = 2, 512, 128
N_CORES = 8
QPC = B * L // N_CORES  # query rows per core (128)
G = 32                  # queries per chunk (softmax granularity)
NCH = QPC // G          # chunks per core (4)
MT = L // 128           # 128-row key tiles per batch (4)


def build_nc() -> bass.Bass:
    # Bacc (not plain Bass): its compile() runs move_matmul_waits_to_ldweights
    # + generate_event_semaphores, which legalize multi-sem waits down to the
    # 1-wait-per-instruction limit this walrus enforces.
    nc = bacc.Bacc()
    F32R = mybir.dt.float32r
    sT_d = nc.declare_dram_parameter("sT", [D, QPC], F32R, isOutput=False)
    hT_d = nc.declare_dram_parameter("hT", [D, L], BF16, isOutput=False)
    hb_d = nc.declare_dram_parameter("hb", [128, MT, D], BF16, isOutput=False)
    W_d = nc.declare_dram_parameter("W", [D, D], F32R, isOutput=False)
    U_d = nc.declare_dram_parameter("U", [D, D], BF16, isOutput=False)
    vm_d = nc.declare_dram_parameter("vmat", [D, G, G], BF16, isOutput=False)
    id_d = nc.declare_dram_parameter("ident", [G, G], BF16, isOutput=False)
    o_d = nc.declare_dram_parameter("out", [QPC, D], F32, isOutput=True)

    with ExitStack() as ctx:
        tc = ctx.enter_context(tile.TileContext(nc))
        consts = ctx.enter_context(tc.tile_pool(name="consts", bufs=1))
        xpool = ctx.enter_context(tc.tile_pool(name="x", bufs=2))
        tpool = ctx.enter_context(tc.tile_pool(name="t", bufs=2))
        spool = ctx.enter_context(tc.tile_pool(name="small", bufs=3))

        # ---------------- prologue ----------------
        # Critical-path loads (hT, U gate the hU matmul -> adds -> 1st tanh)
        # go first on the sync HWDGE queue; bulk non-critical loads go via
        # gpsimd SWDGE so their ~0.6us issue cost doesn't delay the former.
        hT_sb = consts.tile([D, L], BF16)
        nc.sync.dma_start(out=hT_sb, in_=hT_d[:, :])
        U_sb = consts.tile([D, D], BF16)
        nc.sync.dma_start(out=U_sb, in_=U_d[:, :])
        W_sb = consts.tile([D, D], F32R)
        nc.sync.dma_start(out=W_sb, in_=W_d[:, :])
        sT_sb = consts.tile([D, QPC], F32R)
        nc.sync.dma_start(out=sT_sb, in_=sT_d[:, :])
        vmat = consts.tile([D, G, G], BF16)
        nc.gpsimd.dma_start(out=vmat, in_=vm_d[:, :, :])
        hb_sb = consts.tile([128, MT, D], BF16)
        nc.gpsimd.dma_start(out=hb_sb, in_=hb_d[:, :, :])
        ident = consts.tile([G, G], BF16)
        nc.gpsimd.dma_start(out=ident, in_=id_d[:, :])

        hU_sb = consts.tile([D, L], BF16)
        sW_sb = consts.tile([D, QPC], F32)

        with tc.tile_pool(name="pp_pro", bufs=2, space="PSUM") as pp_pro:
            # hU_T[dout, m] = sum_din U[din, dout] * hT[din, m]. float32r
            # streams fp32 data through the PE at 1 cycle/row (vs fp32's
            # two half-speed passes) with TF32-ish multiply precision —
            # plenty here since X is consumed in bf16 anyway.
            hU_ps = pp_pro.tile([D, L], F32, tag="pro")
            nc.tensor.matmul(hU_ps, U_sb, hT_sb, start=True, stop=True)
            nc.vector.tensor_copy(hU_sb, hU_ps)
            sW_ps = pp_pro.tile([D, QPC], F32, tag="pro")
            nc.tensor.matmul(sW_ps, W_sb, sT_sb, start=True, stop=True)
            nc.vector.tensor_copy(sW_sb, sW_ps)

        pp = ctx.enter_context(tc.tile_pool(name="pp", bufs=2, space="PSUM"))

        # ---------------- main loop (software-pipelined emission) --------
        # Per-engine steady-state orders (c = chunk):
        #   ACT: ... tanh(c+1,h0) tanh(c+1,h1) exp(c) ...
        #   DVE: ... adds(c+2) [reduce/recip/eTcopy/scale](c) ...
        #   PE : ... mms(c+1,h0) mms(c+1,h1) [transp/attn](c) ...
        sc_tiles: dict[int, object] = {}
        exp_tiles: dict[int, object] = {}

        def stage_a(c):
            """adds + tanh + v-reduction matmuls for chunk c.

            Chunk 0 uses small (8-query) tanh tiles so the first tanh
            starts as soon as 8 broadcast-adds are done; steady-state
            chunks use 16-query tiles (lower per-instruction overhead).
            """
            if c == 0:
                subs = [8, 8, 8, 8]
            elif c == NCH - 1:
                # small closing tiles: the final exp is gated by the last
                # sub's matmuls, so keep that burst short
                subs = [16, 8, 4, 4]
            else:
                subs = [16, 16]
            sc = pp.tile([G, L], F32, tag="scores")
            j0 = 0
            for si, sub in enumerate(subs):
                X = xpool.tile([D, sub, L], BF16, tag=f"X{si % 2}")
                for j in range(j0, j0 + sub):
                    q = c * G + j
                    nc.vector.tensor_scalar_add(
                        X[:, j - j0, :], hU_sb, sW_sb[:, q : q + 1]
                    )
                T = tpool.tile([D, sub, L], BF16, tag=f"T{si % 2}")
                nc.scalar.activation(
                    T.rearrange("p a b -> p (a b)"),
                    X.rearrange("p a b -> p (a b)"),
                    AF.Tanh,
                )
                for j in range(j0, j0 + sub):
                    nc.tensor.matmul(
                        sc,
                        vmat[:, j, :],
                        T[:, j - j0, :],
                        start=(j == 0),
                        stop=(j == G - 1),
                    )
                j0 += sub
            sc_tiles[c] = sc

        def stage_exp(c):
            """exp for chunk c — emitted after tanh(c+1)."""
            exp_sb = spool.tile([G, L], BF16, tag="exp")
            nc.scalar.activation(exp_sb, sc_tiles.pop(c), AF.Exp)
            exp_tiles[c] = exp_sb

        def stage_tail(c):
            """softmax-normalize + attn @ h + store for chunk c."""
            exp_sb = exp_tiles.pop(c)
            sums = spool.tile([G, 1], F32, tag="sums")
            nc.vector.tensor_reduce(
                sums, exp_sb, axis=mybir.AxisListType.X, op=mybir.AluOpType.add
            )
            recip = spool.tile([G, 1], F32, tag="recip")
            nc.vector.reciprocal(recip, sums)
            eT_ps = pp.tile([128, MT, G], BF16, tag="eT")
            for t in range(MT):
                nc.tensor.transpose(
                    eT_ps[:, t, :], exp_sb[:, t * 128 : (t + 1) * 128], ident
                )
            eT_sb = spool.tile([128, MT, G], BF16, tag="eTs")
            nc.vector.tensor_copy(eT_sb, eT_ps)
            at_ps = pp.tile([G, D], F32, tag="attn")
            for t in range(MT):
                nc.tensor.matmul(
                    at_ps,
                    eT_sb[:, t, :],
                    hb_sb[:, t, :],
                    start=(t == 0),
                    stop=(t == MT - 1),
                )
            out_sb = spool.tile([G, D], F32, tag="out")
            nc.vector.tensor_scalar_mul(out_sb, at_ps, recip[:, 0:1])
            nc.sync.dma_start(out=o_d[bass.ts(c, G), :], in_=out_sb)

        stage_a(0)
        if NCH > 1:
            stage_a(1)
        for c in range(NCH):
            stage_exp(c)
            if c + 2 < NCH:
                stage_a(c + 2)
            stage_tail(c)

    nc.compile()
    return nc


_NC_CACHE: list = []


def _get_nc() -> bass.Bass:
    if not _NC_CACHE:
        _NC_CACHE.append(build_nc())
    return _NC_CACHE[0]


def _make_in_maps(s, h, W, U, v):
    s2 = np.ascontiguousarray(np.asarray(s, np.float32).reshape(B * L, D))
    h2 = np.asarray(h, np.float32)
    W2 = np.ascontiguousarray(np.asarray(W, np.float32))
    U2 = np.ascontiguousarray(np.asarray(U, np.float32))
    v2 = np.asarray(v, np.float32)
    vmat = np.zeros((D, G, G), np.float32)
    for j in range(G):
        vmat[:, j, j] = v2[:, 0]
    vmat = vmat.astype(ml_dtypes.bfloat16)
    ident = np.eye(G, dtype=ml_dtypes.bfloat16)
    in_maps = []
    for c in range(N_CORES):
        b = c * QPC // L
        h_b = h2[b]  # [L, D]
        hb = np.ascontiguousarray(
            h_b.reshape(MT, 128, D).transpose(1, 0, 2).astype(ml_dtypes.bfloat16)
        )
        in_maps.append(
            {
                "sT": np.ascontiguousarray(s2[c * QPC : (c + 1) * QPC].T),
                "hT": np.ascontiguousarray(h_b.T.astype(ml_dtypes.bfloat16)),
                "hb": hb,
                "W": W2,
                "U": np.ascontiguousarray(U2.astype(ml_dtypes.bfloat16)),
                "vmat": vmat,
                "ident": ident,
            }
        )
    return in_maps


def run_spmd(s, h, W, U, v, **kwargs):
    """Run the kernel on 8 cores; returns the BassKernelResults."""
    nc = _get_nc()
    in_maps = _make_in_maps(s, h, W, U, v)
    return run_bass_kernel_spmd(nc, in_maps, core_ids=list(range(N_CORES)), **kwargs)


def kernel(s, h, W, U, v):
    res = run_spmd(s, h, W, U, v)
    shards = [np.asarray(res.results[c]["out"]) for c in range(N_CORES)]
    return np.concatenate(shards, axis=0).reshape(B, L, D).astype(np.float32)
